# revision 5
# baseline (speedup 1.0000x reference)
"""GraphNorm-style segmented normalization on 8 Trainium2 NeuronCores.

Strategy (hardcoded for x:[500000,256] f32, batch:[500000] sorted int,
4096 graphs, alpha/weight/bias:[256]):

- Host: graphs are sorted by size (descending) and dealt round-robin to the
  8 cores, so slot k on every core holds that core's rank-(8k+c) graph and
  is padded to the canonical size S_k = size(rank 8k) (rounded up to even).
  This makes the slot structure (and hence the compiled program) identical
  across cores: one SPMD Bass program, per-core data.
- Host packs each core's nodes channel-major (transposed, [256, Np]) with
  zero padding, so per-graph segment reductions on-device are contiguous
  free-dim slices with compile-time bounds.
- Device (per core, no PE/PSUM, fully DMA-bound design):
    per chunk of slots: DMA load [128, W] x2 channel-tiles ->
    per-slot bn_stats (DVE) -> batched stats math (DVE) using
    E[(x-a*mu)^2] = E[x^2] + (a^2-2a)*mu^2 -> rstd via reciprocal+sqrt ->
    per-slot affine apply out = A*x + B on DVE (tensor_scalar) and ACT
    (activation Identity with per-partition scale/bias) -> DMA store.
- Host unpacks/transposes back and scatters rows to original node order.
"""
import sys

if "/opt/trn_rl_repo" not in sys.path:
    sys.path.insert(0, "/opt/trn_rl_repo")

import numpy as np

import concourse.bacc as bacc
import concourse.tile as tile
from concourse import mybir
from concourse.bass_utils import run_bass_kernel_spmd

F32 = mybir.dt.float32
EPS = 1e-9
N_CORES = 8
H = 256
W_TGT = 4096         # target chunk width (nodes per chunk per channel-tile)
DVE_APPLY_EVERY = 8  # slot k applies on DVE when k % DVE_APPLY_EVERY == 0

_program_cache = {}


def _plan_slots(sizes, n_cores):
    """Assign graphs to (slot, core) and compute canonical padded slot sizes."""
    G = len(sizes)
    Gp = ((G + n_cores - 1) // n_cores) * n_cores
    sizes_p = np.concatenate([sizes, np.zeros(Gp - len(sizes), sizes.dtype)])
    order = np.argsort(-sizes_p, kind="stable")
    ranked = order.reshape(-1, n_cores)          # [M, n_cores] graph ids
    rank_sz = sizes_p[order].reshape(-1, n_cores)
    S = rank_sz[:, 0]                            # canonical size = max in slot
    keep = S > 0
    ranked = ranked[keep]
    S = S[keep].astype(np.int64)
    S = ((S + 1) // 2) * 2                       # even for bn_stats even/odd math
    offs = np.concatenate([[0], np.cumsum(S)])
    return ranked, S, offs


def _plan_chunks(S, w_tgt):
    chunks = []
    k0 = 0
    acc = 0
    for k, s in enumerate(S):
        acc += s
        if acc >= w_tgt:
            chunks.append((k0, k + 1))
            k0 = k + 1
            acc = 0
    if k0 < len(S):
        chunks.append((k0, len(S)))
    return chunks


def _build_program(S, offs, chunks, M, Np):
    nc = bacc.Bacc("TRN2", target_bir_lowering=False, debug=False,
                   num_devices=N_CORES)
    xt_d = nc.dram_tensor("xt", [2, 128, Np], F32, kind="ExternalInput")
    c2_d = nc.dram_tensor("c2", [128, 2, M], F32, kind="ExternalInput")
    c3_d = nc.dram_tensor("c3", [128, 2, M], F32, kind="ExternalInput")
    w_d = nc.dram_tensor("wp", [128, 2], F32, kind="ExternalInput")
    b_d = nc.dram_tensor("bp", [128, 2], F32, kind="ExternalInput")
    caa_d = nc.dram_tensor("caap", [128, 2], F32, kind="ExternalInput")
    nwa_d = nc.dram_tensor("nwap", [128, 2], F32, kind="ExternalInput")
    yt_d = nc.dram_tensor("yt", [2, 128, Np], F32, kind="ExternalOutput")

    mult = mybir.AluOpType.mult
    add = mybir.AluOpType.add

    with tile.TileContext(nc) as tc:
        with (
            tc.tile_pool(name="const", bufs=1) as constp,
            tc.tile_pool(name="xp", bufs=3) as xp,
            tc.tile_pool(name="yp", bufs=2) as yp,
            tc.tile_pool(name="stp", bufs=2) as stp,
            tc.tile_pool(name="abp", bufs=2) as abp,
        ):
            c2t = constp.tile([128, 2, M], F32)
            c3t = constp.tile([128, 2, M], F32)
            wt = constp.tile([128, 2], F32)
            bt = constp.tile([128, 2], F32)
            caat = constp.tile([128, 2], F32)
            nwat = constp.tile([128, 2], F32)
            nc.sync.dma_start(c2t[:], c2_d[:, :, :])
            nc.sync.dma_start(c3t[:], c3_d[:, :, :])
            nc.sync.dma_start(wt[:], w_d[:, :])
            nc.sync.dma_start(bt[:], b_d[:, :])
            nc.sync.dma_start(caat[:], caa_d[:, :])
            nc.sync.dma_start(nwat[:], nwa_d[:, :])

            for (k0, k1) in chunks:
                Mc = k1 - k0
                n0 = int(offs[k0])
                n1 = int(offs[k1])
                Wc = n1 - n0

                X = []
                for t in (0, 1):
                    xtile = xp.tile([128, Wc], F32, tag=f"X{t}")
                    nc.sync.dma_start(xtile[:], xt_d[t, :, n0:n1])
                    X.append(xtile)

                st = stp.tile([128, 2, Mc, 6], F32, tag="st")
                for t in (0, 1):
                    for j in range(Mc):
                        a = int(offs[k0 + j]) - n0
                        s = int(S[k0 + j])
                        nc.vector.bn_stats(st[:, t, j, :], X[t][:, a:a + s])

                # stats math, batched over [128, 2, Mc]
                f1 = st[:, :, :, 1]   # mean of even elements
                f4 = st[:, :, :, 4]   # mean of odd elements
                f2 = st[:, :, :, 2]   # cnt*var even
                f5 = st[:, :, :, 5]   # cnt*var odd
                c2s = c2t[:, :, k0:k1]   # S/(2n)
                c3s = c3t[:, :, k0:k1]   # 1/n

                mu = abp.tile([128, 2, Mc], F32, tag="mu")
                q = abp.tile([128, 2, Mc], F32, tag="q")
                q2 = abp.tile([128, 2, Mc], F32, tag="q2")
                m_ = abp.tile([128, 2, Mc], F32, tag="m_")
                ex2 = abp.tile([128, 2, Mc], F32, tag="ex2")
                sg = abp.tile([128, 2, Mc], F32, tag="sg")
                At = abp.tile([128, 2, Mc], F32, tag="At")
                Bt = abp.tile([128, 2, Mc], F32, tag="Bt")

                v = nc.vector
                v.tensor_tensor(mu[:], f1, f4, add)
                v.tensor_tensor(mu[:], mu[:], c2s, mult)        # mu = (f1+f4)*S/(2n)
                v.tensor_tensor(q[:], f1, f1, mult)
                v.tensor_tensor(q2[:], f4, f4, mult)
                v.tensor_tensor(q[:], q[:], q2[:], add)         # f1^2+f4^2
                v.tensor_tensor(q[:], q[:], c2s, mult)          # *(S/2n)
                v.tensor_tensor(m_[:], f2, f5, add)
                v.tensor_tensor(m_[:], m_[:], c3s, mult)        # (f2+f5)/n
                v.tensor_tensor(ex2[:], m_[:], q[:], add)       # E[x^2]
                v.tensor_tensor(q2[:], mu[:], mu[:], mult)      # mu^2
                # per-channel scalars differ between the two channel-tiles
                for t in (0, 1):
                    # sg = mu^2 * (a^2-2a) + EPS
                    v.tensor_scalar(sg[:, t], q2[:, t], caat[:, t:t + 1],
                                    EPS, mult, add)
                v.tensor_tensor(sg[:], sg[:], ex2[:], add)      # sigma^2 + EPS
                v.reciprocal(sg[:], sg[:])                      # 1/sigma^2
                nc.scalar.sqrt(sg[:], sg[:])                    # rstd (on ACT)
                v.tensor_tensor(Bt[:], mu[:], sg[:], mult)      # mu*rstd
                for t in (0, 1):
                    v.tensor_scalar(At[:, t], sg[:, t], wt[:, t:t + 1],
                                    None, mult)
                    v.tensor_scalar(Bt[:, t], Bt[:, t], nwat[:, t:t + 1],
                                    bt[:, t:t + 1], mult, add)

                # apply per slot
                Y = []
                for t in (0, 1):
                    ytile = yp.tile([128, Wc], F32, tag=f"Y{t}")
                    Y.append(ytile)
                for j in range(Mc):
                    k = k0 + j
                    a = int(offs[k]) - n0
                    s = int(S[k])
                    for t in (0, 1):
                        xs = X[t][:, a:a + s]
                        ys = Y[t][:, a:a + s]
                        Ac = At[:, t, j:j + 1]
                        Bc = Bt[:, t, j:j + 1]
                        if k % DVE_APPLY_EVERY == 0:
                            v.tensor_scalar(ys, xs, Ac, Bc, mult, add)
                        else:
                            nc.scalar.activation(
                                ys, xs, mybir.ActivationFunctionType.Identity,
                                bias=Bc, scale=Ac)
                for t in (0, 1):
                    nc.sync.dma_start(yt_d[t, :, n0:n1], Y[t][:])
    nc.compile()
    return nc


def _build_program_cached(S, offs, chunks, M, Np):
    key = (tuple(int(s) for s in S), tuple(chunks), M, Np)
    nc = _program_cache.get(key)
    if nc is None:
        nc = _build_program(S, offs, chunks, M, Np)
        _program_cache[key] = nc
    return nc


def kernel(x, batch, alpha, weight, bias, num_graphs):
    x = np.asarray(x, dtype=np.float32)
    batch = np.asarray(batch).astype(np.int64)
    alpha = np.asarray(alpha, dtype=np.float32)
    weight = np.asarray(weight, dtype=np.float32)
    bias = np.asarray(bias, dtype=np.float32)
    G = int(num_graphs)
    N, Hx = x.shape
    assert Hx == H

    sizes = np.bincount(batch, minlength=G).astype(np.int64)
    # node rows of graph g (robust to unsorted batch, though spec says sorted)
    node_order = np.argsort(batch, kind="stable")
    gstarts = np.concatenate([[0], np.cumsum(sizes)])

    ranked, S, offs = _plan_slots(sizes, N_CORES)
    M = len(S)
    Np = int(offs[-1])
    chunks = _plan_chunks(S, W_TGT)

    nc = _build_program_cached(S, offs, chunks, M, Np)

    # per-channel parameter vectors, per channel-tile columns [128, 2]
    caa = alpha * alpha - 2.0 * alpha
    nwa = -(weight * alpha)
    w_p = np.ascontiguousarray(weight.reshape(2, 128).T)
    b_p = np.ascontiguousarray(bias.reshape(2, 128).T)
    caa_p = np.ascontiguousarray(caa.reshape(2, 128).T)
    nwa_p = np.ascontiguousarray(nwa.reshape(2, 128).T)

    xa = np.concatenate([x, np.zeros((1, H), np.float32)], axis=0)

    in_maps = []
    idx_per_core = []
    for c in range(N_CORES):
        gids = ranked[:, c]
        n = sizes[gids]
        idx = np.full(Np, N, dtype=np.int64)
        for k in range(M):
            g = gids[k]
            nk = int(n[k])
            if nk:
                idx[int(offs[k]):int(offs[k]) + nk] = \
                    node_order[gstarts[g]:gstarts[g] + nk]
        xp = xa[idx]                                  # [Np, 256] gather
        xt = np.ascontiguousarray(xp.T).reshape(2, 128, Np)
        nguard = np.maximum(n, 1).astype(np.float32)
        c2 = (S.astype(np.float32) / (2.0 * nguard))
        c3 = (1.0 / nguard)
        c2b = np.broadcast_to(c2, (128, 2, M)).astype(np.float32).copy()
        c3b = np.broadcast_to(c3, (128, 2, M)).astype(np.float32).copy()
        in_maps.append({
            "xt": xt, "c2": c2b, "c3": c3b,
            "wp": w_p, "bp": b_p, "caap": caa_p, "nwap": nwa_p,
        })
        idx_per_core.append(idx)
    del xa

    global _last_run
    _last_run = (nc, in_maps)
    res = run_bass_kernel_spmd(nc, in_maps, core_ids=list(range(N_CORES)))

    out = np.empty((N, H), dtype=np.float32)
    for c in range(N_CORES):
        yt = np.asarray(res.results[c]["yt"]).reshape(H, Np)
        yp = np.ascontiguousarray(yt.T)               # [Np, 256]
        idx = idx_per_core[c]
        mask = idx < N
        out[idx[mask]] = yp[mask]
    return out
